# revision 16
# baseline (speedup 1.0000x reference)
"""Trainium2 Bass kernel for nn_AttentionBlock (B=4, H=W=64, C=512).

Strategy (8 cores, no collectives):
  - 2 cores per batch image; each core handles 2048 of the 4096 queries.
  - Key/token order is permuted per core so that each core's OWN query rows
    are tokens 0..2047 of its private x copy (softmax is invariant to key
    permutation as long as K and V use the same order).
  - All GEMMs run in fp8e4 with MatmulPerfMode.DoubleRow (2x bf16 rate):
    lhsT/rhs carry [128, 2, *] channel- or key-chunk pairs so each matmul
    contracts 256 elements. (Dual-fp8 LDWEIGHTS requires a 128-column
    stationary, hence the full-width ones matrix for the sums plane.)
  - Per core: LayerNorm (bn_stats, batched 2 tiles per Ln/Exp), transpose hn
    to channel-major hfT (bf16 PE transpose, fp8 cast on the PSUM->SBUF
    copy), Q^T/K^T (channel-major) + V (token-major) projections in fp8:
        S^T[k,q] = K^T.T @ Q^T     (PSUM fp32)
        P^T = exp(S^T/sqrt(C)-2.9) (ACT, scale+shift folded into the table)
        O^T[c,q] += V.T-pair @ P^T (PSUM planes 0-3, no output transpose)
        sums[q]  += ones.T @ P^T   (PSUM plane 4)
        y = (O^T fp8) proj via 4*Wp back to [q,c]; y *= 1/(4*sums);
        out = y + x + const-biases
  - The scores+exp for the first query tile are computed DURING stage A/B
    (kT/vN chunks become valid in token order), overlapping the copy-bound
    projection phase with ACT exp work; their P^T tiles are cached in SBUF.
  - Epilogue split: sums-row + O^T fp8 casts (ACT) right after the last PV;
    transposes/recip/y-proj deferred past the next tile's first score pairs
    so the PE queue never drains.
  - LN gamma/beta are folded into the QKV weights/biases on the host;
    bv/bp biases are folded into the residual input xr on the host; the
    softmax 1/sqrt(C) scale is applied by the ACT exp instruction.
"""

import os
import sys

import numpy as np
import ml_dtypes

try:
    import concourse.bass as bass
except ImportError:  # pragma: no cover - fresh-dir fallback
    for _p in ("/opt/trn_rl_repo", "/root/.axon_site/_ro/trn_rl_repo"):
        if os.path.isdir(_p) and _p not in sys.path:
            sys.path.insert(0, _p)
    import concourse.bass as bass

import concourse.bacc as bacc
import concourse.tile as tile
from concourse import mybir
from concourse.bass_utils import run_bass_kernel_spmd

F32 = mybir.dt.float32
BF16 = mybir.dt.bfloat16
F8 = mybir.dt.float8e4
AF = mybir.ActivationFunctionType
ALU = mybir.AluOpType
DR = mybir.MatmulPerfMode.DoubleRow
NPF8 = ml_dtypes.float8_e4m3

B, Hh, Ww, C = 4, 64, 64, 512
N_TOK = Hh * Ww          # 4096 tokens per image
NCORES = 8
NQ = N_TOK * B // NCORES  # 2048 queries per core
LN_EPS = 1e-3
CI = C // 128             # 4 channel chunks
SSCALE = 1.0 / float(np.sqrt(np.float32(C)))  # softmax scale, applied in exp
# exp(S*scale + ESHIFT): keeps P <= ~30 and O^T <= ~150 (fp8e4 max 240),
# so the O^T PSUM->SBUF copy is a pure cast. The extra ln(4) is undone by
# 4*Wp and the 4x srow scale (normalization is scale-invariant).
ESHIFT = -(1.5 + float(np.log(4.0)))
WPSCALE = 4.0

LAST_EXEC_NS = None
LAST_RESULT = None


def build_program(n_tok=N_TOK, nq=NQ):
    """Build the per-core Bass program (identical across cores)."""
    assert n_tok % 1024 == 0 and nq % 512 == 0
    nt_tiles = n_tok // 512   # n-tiles for K/V over all tokens
    qt_tiles = nq // 512      # q-tiles for this core's queries
    kc_n = n_tok // 128       # key chunks
    kp_n = kc_n // 2          # key chunk pairs

    nc = bacc.Bacc()
    if os.environ.get("BASS_CACHE_BUST"):
        nc.dram_tensor(f"cachebust_{os.environ['BASS_CACHE_BUST']}", [1, 1], F32)
    x_d = nc.dram_tensor("x", [n_tok, C], F32, kind="ExternalInput")
    xr_d = nc.dram_tensor("xr", [nq, C], F32, kind="ExternalInput")
    wq_d = nc.dram_tensor("wq", [C, C], F8, kind="ExternalInput")
    wk_d = nc.dram_tensor("wk", [C, C], F8, kind="ExternalInput")
    wv_d = nc.dram_tensor("wv", [C, C], F8, kind="ExternalInput")
    wp_d = nc.dram_tensor("wp", [C, C], F8, kind="ExternalInput")
    bq_d = nc.dram_tensor("bq", [128, CI], F32, kind="ExternalInput")
    bk_d = nc.dram_tensor("bk", [128, CI], F32, kind="ExternalInput")
    id_d = nc.dram_tensor("ident", [128, 128], BF16, kind="ExternalInput")
    on_d = nc.dram_tensor("ones", [128, 2, 128], F8, kind="ExternalInput")
    y_d = nc.dram_tensor("y", [nq, C], F32, kind="ExternalOutput")

    # token index mapping: tok = tile*512 + k*128 + p  (p = partition)
    x_re = x_d[:].rearrange("(t k p) c -> p t k c", p=128, k=4)
    xr_re = xr_d[:].rearrange("(t k p) c -> p t k c", p=128, k=4)
    y_re = y_d[:].rearrange("(t k p) c -> p t k c", p=128, k=4)

    from contextlib import ExitStack

    with ExitStack() as ctx:
        tc = ctx.enter_context(tile.TileContext(nc))
        consts = ctx.enter_context(tc.tile_pool(name="consts", bufs=1))
        big = ctx.enter_context(tc.tile_pool(name="big", bufs=1))
        work = ctx.enter_context(tc.tile_pool(name="work", bufs=3))
        stat = ctx.enter_context(tc.tile_pool(name="stat", bufs=4))
        ptp = ctx.enter_context(tc.tile_pool(name="ptp", bufs=4))
        ptc = ctx.enter_context(tc.tile_pool(name="ptc", bufs=kp_n))
        epi = ctx.enter_context(tc.tile_pool(name="epi", bufs=3))
        psS = ctx.enter_context(tc.tile_pool(name="psS", bufs=3, space="PSUM"))

        # ---- first x tile + transpose identity first: they gate the
        # ---- pipeline head; weights are only needed a few µs later.
        x_t0 = work.tile([128, 4, C], F32, tag="x", bufs=6)
        for k in range(4):
            nc.sync.dma_start(out=x_t0[:, k, :], in_=x_re[:, 0, k, :])
        ident = consts.tile([128, 128], BF16)
        nc.sync.dma_start(out=ident, in_=id_d[:])

        # ---- constants ----
        wq_sb = consts.tile([128, CI, C], F8)
        nc.sync.dma_start(out=wq_sb, in_=wq_d[:].rearrange("(ci p) co -> p ci co", p=128))
        wk_sb = consts.tile([128, CI, C], F8)
        nc.sync.dma_start(out=wk_sb, in_=wk_d[:].rearrange("(ci p) co -> p ci co", p=128))
        wv_sb = consts.tile([128, CI, C], F8)
        nc.sync.dma_start(out=wv_sb, in_=wv_d[:].rearrange("(ci p) co -> p ci co", p=128))
        wp_sb = consts.tile([128, CI, C], F8)
        nc.sync.dma_start(out=wp_sb, in_=wp_d[:].rearrange("(ci p) co -> p ci co", p=128))
        bq_sb = consts.tile([128, CI], F32)
        nc.sync.dma_start(out=bq_sb, in_=bq_d[:])
        bk_sb = consts.tile([128, CI], F32)
        nc.sync.dma_start(out=bk_sb, in_=bk_d[:])
        ones8 = consts.tile([128, 2, 128], F8)
        nc.sync.dma_start(out=ones8, in_=on_d[:])
        eps_sb = consts.tile([128, 1], F32)
        nc.vector.memset(eps_sb, LN_EPS)
        shf_sb = consts.tile([128, 1], F32)
        nc.vector.memset(shf_sb, ESHIFT)

        # ---- persistent activations (all fp8, channel pairs sliceable) ----
        hfT = big.tile([128, CI, n_tok], F8)     # normalized x, channel-major
        kT = big.tile([128, CI, n_tok], F8)      # K^T, channel-major
        vN = big.tile([128, kc_n, C], F8)        # V, token-major chunks
        qT = big.tile([128, CI, nq], F8)         # Q^T, channel-major

        # scores + exp for one key chunk; pipelined ahead of PV use
        def st_exp(qt, kc, pt2, plane):
            s_ps = psS.tile([128, 512], F32, tag="st",
                            name=f"s_ps_{qt}_{kc}")
            for ip in range(CI // 2):
                nc.tensor.matmul(
                    s_ps,
                    lhsT=kT[:, 2 * ip:2 * ip + 2,
                            kc * 128:(kc + 1) * 128],
                    rhs=qT[:, 2 * ip:2 * ip + 2,
                           qt * 512:(qt + 1) * 512],
                    perf_mode=DR,
                    start=(ip == 0), stop=(ip == CI // 2 - 1))
            nc.scalar.activation(out=pt2[:, plane, :], in_=s_ps,
                                 func=AF.Exp, scale=SSCALE,
                                 bias=shf_sb)

        def make_pair(qt, p, pool):
            pt2 = pool.tile([128, 2, 512], F8, tag="pt",
                            name=f"pt_{qt}_{p}")
            st_exp(qt, 2 * p, pt2, 0)
            st_exp(qt, 2 * p + 1, pt2, 1)
            return pt2

        # ========= Stage A+B: LN, transpose, projections; the scores+exp
        # ========= for query tile 0 are interleaved as kT chunks land.
        pt0_cache = []
        xtiles = {0: x_t0}

        def fetch_x(t):
            if t not in xtiles and t < nt_tiles:
                xt = work.tile([128, 4, C], F32, tag="x", bufs=6,
                               name=f"x_{t}")
                nc.sync.dma_start(out=xt, in_=x_re[:, t, :, :])
                xtiles[t] = xt

        fetch_x(1)
        with tc.tile_pool(name="psAB", bufs=4, space="PSUM") as psAB:
            for tp in range(nt_tiles // 2):
                # prefetch the NEXT pair's x tiles so the stats chain for
                # tile pair tp+1 never stalls the PE at the boundary
                fetch_x(2 * tp + 2)
                fetch_x(2 * tp + 3)
                xts = [xtiles[2 * tp], xtiles[2 * tp + 1]]
                # batched LN stats: one Ln + one Exp per 8 chunks (2 tiles)
                mv8 = stat.tile([128, 8, 2], F32, tag="mv")
                for ti in range(2):
                    for k in range(4):
                        stats = stat.tile([128, 6], F32, tag="bnst")
                        nc.vector.bn_stats(out=stats, in_=xts[ti][:, k, :])
                        nc.vector.bn_aggr(out=mv8[:, 4 * ti + k, :], in_=stats)
                # rstd = rsqrt(var+eps) entirely on DVE (bitcast seed +
                # two Newton steps): using ACT Ln here would force a
                # 1.3us activation-table swap away from the exp set twice
                # per tile pair.
                I32 = mybir.dt.int32
                veps = stat.tile([128, 8], F32, tag="veps")
                nc.vector.tensor_scalar_add(out=veps, in0=mv8[:, :, 1],
                                            scalar1=LN_EPS)
                yb = stat.tile([128, 8], I32, tag="yb")
                nc.vector.tensor_scalar(out=yb,
                                        in0=veps[:].bitcast(I32),
                                        scalar1=1, scalar2=None,
                                        op0=ALU.logical_shift_right)
                y0b = stat.tile([128, 8], I32, tag="y0b")
                nc.vector.tensor_scalar(out=y0b, in0=yb,
                                        scalar1=0x5f3759df, scalar2=-1,
                                        op0=ALU.subtract, op1=ALU.mult)
                t1 = stat.tile([128, 8], F32, tag="nt1")
                nc.vector.tensor_tensor(out=t1, in0=y0b[:].bitcast(F32),
                                        in1=y0b[:].bitcast(F32), op=ALU.mult)
                t2 = stat.tile([128, 8], F32, tag="nt2")
                nc.vector.tensor_tensor(out=t2, in0=t1, in1=veps, op=ALU.mult)
                t3 = stat.tile([128, 8], F32, tag="nt3")
                nc.vector.tensor_scalar(out=t3, in0=t2, scalar1=-0.5,
                                        scalar2=1.5, op0=ALU.mult,
                                        op1=ALU.add)
                y1 = stat.tile([128, 8], F32, tag="y1")
                nc.vector.tensor_tensor(out=y1, in0=y0b[:].bitcast(F32),
                                        in1=t3, op=ALU.mult)
                u1 = stat.tile([128, 8], F32, tag="nu1")
                nc.vector.tensor_tensor(out=u1, in0=y1, in1=y1, op=ALU.mult)
                u2 = stat.tile([128, 8], F32, tag="nu2")
                nc.vector.tensor_tensor(out=u2, in0=u1, in1=veps, op=ALU.mult)
                u3 = stat.tile([128, 8], F32, tag="nu3")
                nc.vector.tensor_scalar(out=u3, in0=u2, scalar1=-0.5,
                                        scalar2=1.5, op0=ALU.mult,
                                        op1=ALU.add)
                rstd8 = stat.tile([128, 8], F32, tag="rstd")
                nc.vector.tensor_tensor(out=rstd8, in0=y1, in1=u3,
                                        op=ALU.mult)
                for ti in range(2):
                    t = 2 * tp + ti
                    x_t = xts[ti]
                    for k in range(4):
                        chunk = t * 4 + k
                        hn = work.tile([128, C], BF16, tag="hn", bufs=4)
                        nc.vector.tensor_scalar(out=hn, in0=x_t[:, k, :],
                                                scalar1=mv8[:, 4 * ti + k, 0:1],
                                                scalar2=rstd8[:, 4 * ti + k:4 * ti + k + 1],
                                                op0=ALU.subtract, op1=ALU.mult)
                        tr_ps = psAB.tile([128, CI, 128], BF16, tag="ps")
                        for j in range(CI):
                            nc.tensor.transpose(tr_ps[:, j, :],
                                                hn[:, j * 128:(j + 1) * 128],
                                                ident)
                        # fp8 cast on the PSUM->SBUF copy
                        if k % 2 == 0:
                            nc.scalar.copy(
                                out=hfT[:, :, chunk * 128:(chunk + 1) * 128],
                                in_=tr_ps)
                        else:
                            nc.vector.tensor_copy(
                                out=hfT[:, :, chunk * 128:(chunk + 1) * 128],
                                in_=tr_ps)

                    # V rows for this tile's 4 chunks (DoubleRow, ci pairs)
                    for k in range(4):
                        chunk = t * 4 + k
                        v_ps = psAB.tile([128, C], F32, tag="ps")
                        for ip in range(CI // 2):
                            nc.tensor.matmul(
                                v_ps,
                                lhsT=hfT[:, 2 * ip:2 * ip + 2,
                                         chunk * 128:(chunk + 1) * 128],
                                rhs=wv_sb[:, 2 * ip:2 * ip + 2, :],
                                perf_mode=DR,
                                start=(ip == 0), stop=(ip == CI // 2 - 1))
                        if k % 2 == 0:
                            nc.scalar.copy(out=vN[:, chunk, :], in_=v_ps)
                        else:
                            nc.vector.tensor_copy(out=vN[:, chunk, :],
                                                  in_=v_ps)

                    # K^T columns for this tile
                    for j in range(CI):
                        k_ps = psAB.tile([128, 512], F32, tag="ps")
                        for ip in range(CI // 2):
                            nc.tensor.matmul(
                                k_ps,
                                lhsT=wk_sb[:, 2 * ip:2 * ip + 2,
                                           j * 128:(j + 1) * 128],
                                rhs=hfT[:, 2 * ip:2 * ip + 2,
                                        t * 512:(t + 1) * 512],
                                perf_mode=DR,
                                start=(ip == 0), stop=(ip == CI // 2 - 1))
                        if j % 2 == 0:
                            nc.scalar.activation(
                                out=kT[:, j, t * 512:(t + 1) * 512],
                                in_=k_ps, func=AF.Identity,
                                bias=bk_sb[:, j:j + 1])
                        else:
                            nc.vector.tensor_scalar_add(
                                out=kT[:, j, t * 512:(t + 1) * 512],
                                in0=k_ps, scalar1=bk_sb[:, j:j + 1])

                    # Q^T columns (only for this core's query range)
                    if t < qt_tiles:
                        for j in range(CI):
                            q_ps = psAB.tile([128, 512], F32, tag="ps")
                            for ip in range(CI // 2):
                                nc.tensor.matmul(
                                    q_ps,
                                    lhsT=wq_sb[:, 2 * ip:2 * ip + 2,
                                               j * 128:(j + 1) * 128],
                                    rhs=hfT[:, 2 * ip:2 * ip + 2,
                                            t * 512:(t + 1) * 512],
                                    perf_mode=DR,
                                    start=(ip == 0), stop=(ip == CI // 2 - 1))
                            if j % 2 == 0:
                                nc.scalar.activation(
                                    out=qT[:, j, t * 512:(t + 1) * 512],
                                    in_=q_ps, func=AF.Identity,
                                    bias=bq_sb[:, j:j + 1])
                            else:
                                nc.vector.tensor_scalar_add(
                                    out=qT[:, j, t * 512:(t + 1) * 512],
                                    in0=q_ps, scalar1=bq_sb[:, j:j + 1])

                # prebuild query-tile-0 score pairs for this tp's kc range
                for p in range(4 * tp, 4 * tp + 4):
                    pt0_cache.append(make_pair(0, p, ptc))

        # ================= Stage C: attention ============================
        with tc.tile_pool(name="psO", bufs=1, space="PSUM") as psO:

            # part A: drain O^T/sums out of PSUM (ACT); emitted right after
            # the last PV so the next tile's oT_ps reallocation is safe.
            def epilogue_a(qt, oT_ps):
                srow = epi.tile([1, 512], BF16, tag="srow",
                                name=f"srow_{qt}")
                # 4x so recip = 1/(4*sums) matches y_ps = o8 @ (4*Wp)
                nc.scalar.mul(srow, oT_ps[0:1, 4, :], WPSCALE)
                oT8 = epi.tile([128, 4, C], F8, tag="ot8", name=f"ot8_{qt}")
                for cj in range(4):  # pure casts: one per PSUM bank
                    nc.scalar.copy(out=oT8[:, cj, :], in_=oT_ps[:, cj, :])
                return srow, oT8

            # part B: transposes/recip/y-projection; deferred past the next
            # tile's first score pairs so PE always has queued work.
            def epilogue_b(qt, xr_t, srow, oT8):
                # bf16 PSUM writes need 4-byte alignment -> stride-2 columns
                st4 = psS.tile([128, 4, 2], BF16, tag="st",
                               name=f"st4_{qt}")
                for i in range(4):
                    nc.tensor.transpose(st4[:, i, 0:1],
                                        srow[0:1, i * 128:(i + 1) * 128],
                                        ident[0:1, 0:1])
                recip = stat.tile([128, 4], F32, tag="recip",
                                  name=f"recip_{qt}")
                nc.vector.reciprocal(out=recip, in_=st4[:, :, 0])
                for qc in range(4):
                    y_ps = psS.tile([128, C], F32, tag="st",
                                    name=f"y_ps_{qt}_{qc}")
                    for ip in range(CI // 2):
                        nc.tensor.matmul(
                            y_ps,
                            lhsT=oT8[:, 2 * ip:2 * ip + 2,
                                     qc * 128:(qc + 1) * 128],
                            rhs=wp_sb[:, 2 * ip:2 * ip + 2, :],
                            perf_mode=DR,
                            start=(ip == 0), stop=(ip == CI // 2 - 1))
                    y_sb = epi.tile([128, C], F32, tag="ysb")
                    nc.vector.scalar_tensor_tensor(
                        out=y_sb, in0=y_ps, scalar=recip[:, qc:qc + 1],
                        in1=xr_t[:, qc, :], op0=ALU.mult, op1=ALU.add)
                    nc.sync.dma_start(out=y_re[:, qt, qc, :], in_=y_sb)

            pending = None
            for qt in range(qt_tiles):
                xr_t = work.tile([128, 4, C], F32, tag="xr")
                nc.sync.dma_start(out=xr_t, in_=xr_re[:, qt, :, :])
                if qt == 0:
                    pts = dict(enumerate(pt0_cache))
                else:
                    pts = {0: make_pair(qt, 0, ptp),
                           1: make_pair(qt, 1, ptp)}
                if pending is not None:
                    epilogue_b(*pending)
                    pending = None
                # planes 0-3: O^T[cj]; plane 4: per-query sums (replicated)
                oT_ps = psO.tile([128, 5, C], F32, tag="o", name=f"o_{qt}")
                for p in range(kp_n):
                    if qt != 0 and p + 2 < kp_n:
                        pts[p + 2] = make_pair(qt, p + 2, ptp)
                    pt2 = pts.pop(p)
                    for cj in range(4):
                        nc.tensor.matmul(
                            oT_ps[:, cj, :],
                            lhsT=vN[:, 2 * p:2 * p + 2,
                                    cj * 128:(cj + 1) * 128],
                            rhs=pt2[:],
                            perf_mode=DR,
                            start=(p == 0), stop=(p == kp_n - 1))
                    nc.tensor.matmul(
                        oT_ps[:, 4, :],
                        lhsT=ones8,
                        rhs=pt2[:],
                        perf_mode=DR,
                        start=(p == 0), stop=(p == kp_n - 1))
                srow, oT8 = epilogue_a(qt, oT_ps)
                pending = (qt, xr_t, srow, oT8)
            epilogue_b(*pending)

    nc.compile()
    return nc


def prep_host_inputs(x, ln_g, ln_b, Wq, bq, Wk, bk, Wv, bv, Wp, bp,
                     n_tok=N_TOK, nq=NQ, ncores=NCORES, nbatch=B):
    """Fold LN affine + linear biases on the host; build per-core maps."""
    f32 = np.float32
    x = np.asarray(x, f32)
    g = np.asarray(ln_g, f32)
    b = np.asarray(ln_b, f32)
    Wq = np.asarray(Wq, f32); Wk = np.asarray(Wk, f32)
    Wv = np.asarray(Wv, f32); Wp = np.asarray(Wp, f32)
    bq = np.asarray(bq, f32); bk = np.asarray(bk, f32)
    bv = np.asarray(bv, f32); bp = np.asarray(bp, f32)

    wq_e = g[:, None] * Wq
    bq_e = b @ Wq + bq
    wk_e = g[:, None] * Wk
    bk_e = b @ Wk + bk
    wv_e = g[:, None] * Wv
    bv_e = b @ Wv + bv
    resid_const = bv_e @ Wp + bp    # [C]

    ci = C // 128
    bq_pp = np.ascontiguousarray(bq_e.reshape(ci, 128).T).astype(f32)
    bk_pp = np.ascontiguousarray(bk_e.reshape(ci, 128).T).astype(f32)
    ident = np.eye(128, dtype=ml_dtypes.bfloat16)
    onesv = np.ones((128, 2, 128), dtype=NPF8)

    shared = dict(
        wq=wq_e.astype(NPF8), wk=wk_e.astype(NPF8),
        wv=wv_e.astype(NPF8), wp=(Wp * WPSCALE).astype(NPF8),
        bq=bq_pp, bk=bk_pp, ident=ident, ones=onesv,
    )

    xf = x.reshape(-1, C)  # flattened tokens, nbatch * n_tok rows
    halves = ncores // nbatch
    in_maps = []
    for core in range(ncores):
        bidx, half = divmod(core, halves)
        xb = xf[bidx * n_tok:(bidx + 1) * n_tok]
        if half:
            xp = np.ascontiguousarray(
                np.concatenate([xb[half * nq:], xb[:half * nq]], axis=0))
        else:
            xp = xb
        xr = (xp[:nq] + resid_const).astype(f32)
        m = dict(shared)
        m["x"] = np.ascontiguousarray(xp)
        m["xr"] = np.ascontiguousarray(xr)
        in_maps.append(m)
    return in_maps


_PROG = None


def _get_prog():
    global _PROG
    if _PROG is None:
        _PROG = build_program()
    return _PROG


def kernel(x, ln_g, ln_b, Wq, bq, Wk, bk, Wv, bv, Wp, bp, _trace=False,
           _tmpdir=None):
    global LAST_EXEC_NS, LAST_RESULT
    nc = _get_prog()
    in_maps = prep_host_inputs(x, ln_g, ln_b, Wq, bq, Wk, bk, Wv, bv, Wp, bp)
    res = run_bass_kernel_spmd(nc, in_maps, list(range(NCORES)), trace=_trace,
                               tmpdir=_tmpdir)
    LAST_EXEC_NS = res.exec_time_ns
    LAST_RESULT = res
    y = np.empty((B, N_TOK, C), np.float32)
    halves = NCORES // B
    for core in range(NCORES):
        bidx, half = divmod(core, halves)
        y[bidx, half * NQ:(half + 1) * NQ] = res.results[core]["y"]
    return y.reshape(B, Hh, Ww, C)


# revision 17
# speedup vs baseline: 1.0067x; 1.0067x over previous
"""Trainium2 Bass kernel for nn_AttentionBlock (B=4, H=W=64, C=512).

Strategy (8 cores, no collectives):
  - 2 cores per batch image; each core handles 2048 of the 4096 queries.
  - Key/token order is permuted per core so that each core's OWN query rows
    are tokens 0..2047 of its private x copy (softmax is invariant to key
    permutation as long as K and V use the same order).
  - All GEMMs run in fp8e4 with MatmulPerfMode.DoubleRow (2x bf16 rate):
    lhsT/rhs carry [128, 2, *] channel- or key-chunk pairs so each matmul
    contracts 256 elements. (Dual-fp8 LDWEIGHTS requires a 128-column
    stationary, hence the full-width ones matrix for the sums plane.)
  - Per core: LayerNorm (bn_stats, batched 2 tiles per Ln/Exp), transpose hn
    to channel-major hfT (bf16 PE transpose, fp8 cast on the PSUM->SBUF
    copy), Q^T/K^T (channel-major) + V (token-major) projections in fp8:
        S^T[k,q] = K^T.T @ Q^T     (PSUM fp32)
        P^T = exp(S^T/sqrt(C)-2.9) (ACT, scale+shift folded into the table)
        O^T[c,q] += V.T-pair @ P^T (PSUM planes 0-3, no output transpose)
        sums[q]  += ones.T @ P^T   (PSUM plane 4)
        y = (O^T fp8) proj via 4*Wp back to [q,c]; y *= 1/(4*sums);
        out = y + x + const-biases
  - The scores+exp for the first query tile are computed DURING stage A/B
    (kT/vN chunks become valid in token order), overlapping the copy-bound
    projection phase with ACT exp work; their P^T tiles are cached in SBUF.
  - Epilogue split: sums-row + O^T fp8 casts (ACT) right after the last PV;
    transposes/recip/y-proj deferred past the next tile's first score pairs
    so the PE queue never drains.
  - LN gamma/beta are folded into the QKV weights/biases on the host;
    bv/bp biases are folded into the residual input xr on the host; the
    softmax 1/sqrt(C) scale is applied by the ACT exp instruction.
"""

import os
import sys

import numpy as np
import ml_dtypes

try:
    import concourse.bass as bass
except ImportError:  # pragma: no cover - fresh-dir fallback
    for _p in ("/opt/trn_rl_repo", "/root/.axon_site/_ro/trn_rl_repo"):
        if os.path.isdir(_p) and _p not in sys.path:
            sys.path.insert(0, _p)
    import concourse.bass as bass

import concourse.bacc as bacc
import concourse.tile as tile
from concourse import mybir
from concourse.bass_utils import run_bass_kernel_spmd

F32 = mybir.dt.float32
BF16 = mybir.dt.bfloat16
F8 = mybir.dt.float8e4
AF = mybir.ActivationFunctionType
ALU = mybir.AluOpType
DR = mybir.MatmulPerfMode.DoubleRow
NPF8 = ml_dtypes.float8_e4m3

B, Hh, Ww, C = 4, 64, 64, 512
N_TOK = Hh * Ww          # 4096 tokens per image
NCORES = 8
NQ = N_TOK * B // NCORES  # 2048 queries per core
LN_EPS = 1e-3
CI = C // 128             # 4 channel chunks
SSCALE = 1.0 / float(np.sqrt(np.float32(C)))  # softmax scale, applied in exp
# exp(S*scale + ESHIFT): keeps P <= ~30 and O^T <= ~150 (fp8e4 max 240),
# so the O^T PSUM->SBUF copy is a pure cast. The extra ln(4) is undone by
# 4*Wp and the 4x srow scale (normalization is scale-invariant).
ESHIFT = -(1.5 + float(np.log(4.0)))
WPSCALE = 4.0

LAST_EXEC_NS = None
LAST_RESULT = None


def build_program(n_tok=N_TOK, nq=NQ):
    """Build the per-core Bass program (identical across cores)."""
    assert n_tok % 1024 == 0 and nq % 512 == 0
    nt_tiles = n_tok // 512   # n-tiles for K/V over all tokens
    qt_tiles = nq // 512      # q-tiles for this core's queries
    kc_n = n_tok // 128       # key chunks
    kp_n = kc_n // 2          # key chunk pairs

    nc = bacc.Bacc()
    if os.environ.get("BASS_CACHE_BUST"):
        nc.dram_tensor(f"cachebust_{os.environ['BASS_CACHE_BUST']}", [1, 1], F32)
    x_d = nc.dram_tensor("x", [n_tok, C], F32, kind="ExternalInput")
    xr_d = nc.dram_tensor("xr", [nq, C], F32, kind="ExternalInput")
    wq_d = nc.dram_tensor("wq", [C, C], F8, kind="ExternalInput")
    wk_d = nc.dram_tensor("wk", [C, C], F8, kind="ExternalInput")
    wv_d = nc.dram_tensor("wv", [C, C], F8, kind="ExternalInput")
    wp_d = nc.dram_tensor("wp", [C, C], F8, kind="ExternalInput")
    bq_d = nc.dram_tensor("bq", [128, CI], F32, kind="ExternalInput")
    bk_d = nc.dram_tensor("bk", [128, CI], F32, kind="ExternalInput")
    id_d = nc.dram_tensor("ident", [128, 128], BF16, kind="ExternalInput")
    on_d = nc.dram_tensor("ones", [128, 2, 128], F8, kind="ExternalInput")
    y_d = nc.dram_tensor("y", [nq, C], F32, kind="ExternalOutput")

    # token index mapping: tok = tile*512 + k*128 + p  (p = partition)
    x_re = x_d[:].rearrange("(t k p) c -> p t k c", p=128, k=4)
    xr_re = xr_d[:].rearrange("(t k p) c -> p t k c", p=128, k=4)
    y_re = y_d[:].rearrange("(t k p) c -> p t k c", p=128, k=4)

    from contextlib import ExitStack

    with ExitStack() as ctx:
        tc = ctx.enter_context(tile.TileContext(nc))
        consts = ctx.enter_context(tc.tile_pool(name="consts", bufs=1))
        big = ctx.enter_context(tc.tile_pool(name="big", bufs=1))
        work = ctx.enter_context(tc.tile_pool(name="work", bufs=3))
        stat = ctx.enter_context(tc.tile_pool(name="stat", bufs=4))
        ptp = ctx.enter_context(tc.tile_pool(name="ptp", bufs=4))
        ptc = ctx.enter_context(tc.tile_pool(name="ptc", bufs=kp_n))
        epi = ctx.enter_context(tc.tile_pool(name="epi", bufs=3))
        psS = ctx.enter_context(tc.tile_pool(name="psS", bufs=3, space="PSUM"))

        # ---- first x tile + transpose identity first: they gate the
        # ---- pipeline head; weights are only needed a few µs later.
        x_t0 = work.tile([128, 4, C], F32, tag="x", bufs=6)
        for k in range(4):
            nc.sync.dma_start(out=x_t0[:, k, :], in_=x_re[:, 0, k, :])
        ident = consts.tile([128, 128], BF16)
        nc.sync.dma_start(out=ident, in_=id_d[:])

        # ---- constants ----
        wq_sb = consts.tile([128, CI, C], F8)
        nc.sync.dma_start(out=wq_sb, in_=wq_d[:].rearrange("(ci p) co -> p ci co", p=128))
        wk_sb = consts.tile([128, CI, C], F8)
        nc.sync.dma_start(out=wk_sb, in_=wk_d[:].rearrange("(ci p) co -> p ci co", p=128))
        wv_sb = consts.tile([128, CI, C], F8)
        nc.sync.dma_start(out=wv_sb, in_=wv_d[:].rearrange("(ci p) co -> p ci co", p=128))
        wp_sb = consts.tile([128, CI, C], F8)
        nc.sync.dma_start(out=wp_sb, in_=wp_d[:].rearrange("(ci p) co -> p ci co", p=128))
        bq_sb = consts.tile([128, CI], F32)
        nc.sync.dma_start(out=bq_sb, in_=bq_d[:])
        bk_sb = consts.tile([128, CI], F32)
        nc.sync.dma_start(out=bk_sb, in_=bk_d[:])
        ones8 = consts.tile([128, 2, 128], F8)
        nc.sync.dma_start(out=ones8, in_=on_d[:])
        eps_sb = consts.tile([128, 1], F32)
        nc.vector.memset(eps_sb, LN_EPS)
        shf_sb = consts.tile([128, 1], F32)
        nc.vector.memset(shf_sb, ESHIFT)

        # ---- persistent activations (all fp8, channel pairs sliceable) ----
        hfT = big.tile([128, CI, n_tok], F8)     # normalized x, channel-major
        kT = big.tile([128, CI, n_tok], F8)      # K^T, channel-major
        vN = big.tile([128, kc_n, C], F8)        # V, token-major chunks
        qT = big.tile([128, CI, nq], F8)         # Q^T, channel-major

        # scores + exp for one key chunk; pipelined ahead of PV use
        def st_exp(qt, kc, pt2, plane):
            s_ps = psS.tile([128, 512], F32, tag="st",
                            name=f"s_ps_{qt}_{kc}")
            for ip in range(CI // 2):
                nc.tensor.matmul(
                    s_ps,
                    lhsT=kT[:, 2 * ip:2 * ip + 2,
                            kc * 128:(kc + 1) * 128],
                    rhs=qT[:, 2 * ip:2 * ip + 2,
                           qt * 512:(qt + 1) * 512],
                    perf_mode=DR,
                    start=(ip == 0), stop=(ip == CI // 2 - 1))
            nc.scalar.activation(out=pt2[:, plane, :], in_=s_ps,
                                 func=AF.Exp, scale=SSCALE,
                                 bias=shf_sb)

        def make_pair(qt, p, pool):
            pt2 = pool.tile([128, 2, 512], F8, tag="pt",
                            name=f"pt_{qt}_{p}")
            st_exp(qt, 2 * p, pt2, 0)
            st_exp(qt, 2 * p + 1, pt2, 1)
            return pt2

        # ========= Stage A+B: LN, transpose, projections; the scores+exp
        # ========= for query tile 0 are interleaved as kT chunks land.
        pt0_cache = []
        xtiles = {0: x_t0}

        def fetch_x(t):
            if t not in xtiles and t < nt_tiles:
                xt = work.tile([128, 4, C], F32, tag="x", bufs=6,
                               name=f"x_{t}")
                nc.sync.dma_start(out=xt, in_=x_re[:, t, :, :])
                xtiles[t] = xt

        fetch_x(1)

        def emit_stats(tp):
            """bn_stats + DVE fast-rsqrt for tile pair tp -> (rstd8, b8)."""
            xts = [xtiles[2 * tp], xtiles[2 * tp + 1]]
            mv8 = stat.tile([128, 8, 2], F32, tag="mv", name=f"mv_{tp}")
            for ti in range(2):
                for k in range(4):
                    stats = stat.tile([128, 6], F32, tag="bnst")
                    nc.vector.bn_stats(out=stats, in_=xts[ti][:, k, :])
                    nc.vector.bn_aggr(out=mv8[:, 4 * ti + k, :], in_=stats)
            # rstd = rsqrt(var+eps) entirely on DVE (bitcast seed + two
            # Newton steps): ACT Ln/Sqrt would force a 1.3us activation-
            # table swap away from the exp set.
            I32 = mybir.dt.int32
            veps = stat.tile([128, 8], F32, tag="veps")
            nc.vector.tensor_scalar_add(out=veps, in0=mv8[:, :, 1],
                                        scalar1=LN_EPS)
            yb = stat.tile([128, 8], I32, tag="yb")
            nc.vector.tensor_scalar(out=yb,
                                    in0=veps[:].bitcast(I32),
                                    scalar1=1, scalar2=None,
                                    op0=ALU.logical_shift_right)
            y0b = stat.tile([128, 8], I32, tag="y0b")
            nc.vector.tensor_scalar(out=y0b, in0=yb,
                                    scalar1=0x5f3759df, scalar2=-1,
                                    op0=ALU.subtract, op1=ALU.mult)
            t1 = stat.tile([128, 8], F32, tag="nt1")
            nc.vector.tensor_tensor(out=t1, in0=y0b[:].bitcast(F32),
                                    in1=y0b[:].bitcast(F32), op=ALU.mult)
            t2 = stat.tile([128, 8], F32, tag="nt2")
            nc.vector.tensor_tensor(out=t2, in0=t1, in1=veps, op=ALU.mult)
            t3 = stat.tile([128, 8], F32, tag="nt3")
            nc.vector.tensor_scalar(out=t3, in0=t2, scalar1=-0.5,
                                    scalar2=1.5, op0=ALU.mult, op1=ALU.add)
            y1 = stat.tile([128, 8], F32, tag="y1")
            nc.vector.tensor_tensor(out=y1, in0=y0b[:].bitcast(F32),
                                    in1=t3, op=ALU.mult)
            u1 = stat.tile([128, 8], F32, tag="nu1")
            nc.vector.tensor_tensor(out=u1, in0=y1, in1=y1, op=ALU.mult)
            u2 = stat.tile([128, 8], F32, tag="nu2")
            nc.vector.tensor_tensor(out=u2, in0=u1, in1=veps, op=ALU.mult)
            u3 = stat.tile([128, 8], F32, tag="nu3")
            nc.vector.tensor_scalar(out=u3, in0=u2, scalar1=-0.5,
                                    scalar2=1.5, op0=ALU.mult, op1=ALU.add)
            rstd8 = stat.tile([128, 8], F32, tag="rstd", name=f"rstd_{tp}")
            nc.vector.tensor_tensor(out=rstd8, in0=y1, in1=u3, op=ALU.mult)
            # hn is applied on ACT as Identity(x*rstd + (-mu*rstd))
            mr = stat.tile([128, 8], F32, tag="mr")
            nc.vector.tensor_tensor(out=mr, in0=mv8[:, :, 0], in1=rstd8,
                                    op=ALU.mult)
            b8 = stat.tile([128, 8], F32, tag="b8", name=f"b8_{tp}")
            nc.vector.tensor_scalar_mul(out=b8, in0=mr, scalar1=-1.0)
            return rstd8, b8

        ln_aff = {}
        with tc.tile_pool(name="psAB", bufs=4, space="PSUM") as psAB:
            for tp in range(nt_tiles // 2):
                # prefetch the NEXT pair's x tiles so the stats chain for
                # tile pair tp+1 never stalls the PE at the boundary
                fetch_x(2 * tp + 2)
                fetch_x(2 * tp + 3)
                if tp == 0:
                    ln_aff[0] = emit_stats(0)
                rstd8, b8 = ln_aff.pop(tp)
                for ti in range(2):
                    t = 2 * tp + ti
                    x_t = xtiles[t]
                    # emit next pair's stats midway: its x DMAs have landed
                    # and the DVE reaches it before the boundary
                    if ti == 1 and tp + 1 < nt_tiles // 2:
                        ln_aff[tp + 1] = emit_stats(tp + 1)
                    for k in range(4):
                        chunk = t * 4 + k
                        idx = 4 * ti + k
                        hn = work.tile([128, C], BF16, tag="hn", bufs=4)
                        nc.scalar.activation(out=hn, in_=x_t[:, k, :],
                                             func=AF.Identity,
                                             scale=rstd8[:, idx:idx + 1],
                                             bias=b8[:, idx:idx + 1])
                        tr_ps = psAB.tile([128, CI, 128], BF16, tag="ps")
                        for j in range(CI):
                            nc.tensor.transpose(tr_ps[:, j, :],
                                                hn[:, j * 128:(j + 1) * 128],
                                                ident)
                        # fp8 cast on the PSUM->SBUF copy
                        nc.vector.tensor_copy(
                            out=hfT[:, :, chunk * 128:(chunk + 1) * 128],
                            in_=tr_ps)

                    # V rows for this tile's 4 chunks (DoubleRow, ci pairs)
                    for k in range(4):
                        chunk = t * 4 + k
                        v_ps = psAB.tile([128, C], F32, tag="ps")
                        for ip in range(CI // 2):
                            nc.tensor.matmul(
                                v_ps,
                                lhsT=hfT[:, 2 * ip:2 * ip + 2,
                                         chunk * 128:(chunk + 1) * 128],
                                rhs=wv_sb[:, 2 * ip:2 * ip + 2, :],
                                perf_mode=DR,
                                start=(ip == 0), stop=(ip == CI // 2 - 1))
                        if k % 2 == 0:
                            nc.scalar.copy(out=vN[:, chunk, :], in_=v_ps)
                        else:
                            nc.vector.tensor_copy(out=vN[:, chunk, :],
                                                  in_=v_ps)

                    # K^T columns for this tile
                    for j in range(CI):
                        k_ps = psAB.tile([128, 512], F32, tag="ps")
                        for ip in range(CI // 2):
                            nc.tensor.matmul(
                                k_ps,
                                lhsT=wk_sb[:, 2 * ip:2 * ip + 2,
                                           j * 128:(j + 1) * 128],
                                rhs=hfT[:, 2 * ip:2 * ip + 2,
                                        t * 512:(t + 1) * 512],
                                perf_mode=DR,
                                start=(ip == 0), stop=(ip == CI // 2 - 1))
                        if j % 2 == 0:
                            nc.scalar.activation(
                                out=kT[:, j, t * 512:(t + 1) * 512],
                                in_=k_ps, func=AF.Identity,
                                bias=bk_sb[:, j:j + 1])
                        else:
                            nc.vector.tensor_scalar_add(
                                out=kT[:, j, t * 512:(t + 1) * 512],
                                in0=k_ps, scalar1=bk_sb[:, j:j + 1])

                    # Q^T columns (only for this core's query range)
                    if t < qt_tiles:
                        for j in range(CI):
                            q_ps = psAB.tile([128, 512], F32, tag="ps")
                            for ip in range(CI // 2):
                                nc.tensor.matmul(
                                    q_ps,
                                    lhsT=wq_sb[:, 2 * ip:2 * ip + 2,
                                               j * 128:(j + 1) * 128],
                                    rhs=hfT[:, 2 * ip:2 * ip + 2,
                                            t * 512:(t + 1) * 512],
                                    perf_mode=DR,
                                    start=(ip == 0), stop=(ip == CI // 2 - 1))
                            if j % 2 == 0:
                                nc.scalar.activation(
                                    out=qT[:, j, t * 512:(t + 1) * 512],
                                    in_=q_ps, func=AF.Identity,
                                    bias=bq_sb[:, j:j + 1])
                            else:
                                nc.vector.tensor_scalar_add(
                                    out=qT[:, j, t * 512:(t + 1) * 512],
                                    in0=q_ps, scalar1=bq_sb[:, j:j + 1])

                # prebuild query-tile-0 score pairs for this tp's kc range
                for p in range(4 * tp, 4 * tp + 4):
                    pt0_cache.append(make_pair(0, p, ptc))

        # ================= Stage C: attention ============================
        with tc.tile_pool(name="psO", bufs=1, space="PSUM") as psO:

            # part A: drain O^T/sums out of PSUM (ACT); emitted right after
            # the last PV so the next tile's oT_ps reallocation is safe.
            def epilogue_a(qt, oT_ps):
                srow = epi.tile([1, 512], BF16, tag="srow",
                                name=f"srow_{qt}")
                # 4x so recip = 1/(4*sums) matches y_ps = o8 @ (4*Wp)
                nc.scalar.mul(srow, oT_ps[0:1, 4, :], WPSCALE)
                oT8 = epi.tile([128, 4, C], F8, tag="ot8", name=f"ot8_{qt}")
                for cj in range(4):  # pure casts: one per PSUM bank
                    nc.scalar.copy(out=oT8[:, cj, :], in_=oT_ps[:, cj, :])
                return srow, oT8

            # part B: transposes/recip/y-projection; deferred past the next
            # tile's first score pairs so PE always has queued work.
            def epilogue_b(qt, xr_t, srow, oT8):
                # bf16 PSUM writes need 4-byte alignment -> stride-2 columns
                st4 = psS.tile([128, 4, 2], BF16, tag="st",
                               name=f"st4_{qt}")
                for i in range(4):
                    nc.tensor.transpose(st4[:, i, 0:1],
                                        srow[0:1, i * 128:(i + 1) * 128],
                                        ident[0:1, 0:1])
                recip = stat.tile([128, 4], F32, tag="recip",
                                  name=f"recip_{qt}")
                nc.vector.reciprocal(out=recip, in_=st4[:, :, 0])
                for qc in range(4):
                    y_ps = psS.tile([128, C], F32, tag="st",
                                    name=f"y_ps_{qt}_{qc}")
                    for ip in range(CI // 2):
                        nc.tensor.matmul(
                            y_ps,
                            lhsT=oT8[:, 2 * ip:2 * ip + 2,
                                     qc * 128:(qc + 1) * 128],
                            rhs=wp_sb[:, 2 * ip:2 * ip + 2, :],
                            perf_mode=DR,
                            start=(ip == 0), stop=(ip == CI // 2 - 1))
                    y_sb = epi.tile([128, C], F32, tag="ysb")
                    nc.vector.scalar_tensor_tensor(
                        out=y_sb, in0=y_ps, scalar=recip[:, qc:qc + 1],
                        in1=xr_t[:, qc, :], op0=ALU.mult, op1=ALU.add)
                    nc.sync.dma_start(out=y_re[:, qt, qc, :], in_=y_sb)

            pending = None
            for qt in range(qt_tiles):
                xr_t = work.tile([128, 4, C], F32, tag="xr")
                nc.sync.dma_start(out=xr_t, in_=xr_re[:, qt, :, :])
                if qt == 0:
                    pts = dict(enumerate(pt0_cache))
                else:
                    pts = {0: make_pair(qt, 0, ptp),
                           1: make_pair(qt, 1, ptp)}
                if pending is not None:
                    epilogue_b(*pending)
                    pending = None
                # planes 0-3: O^T[cj]; plane 4: per-query sums (replicated)
                oT_ps = psO.tile([128, 5, C], F32, tag="o", name=f"o_{qt}")
                for p in range(kp_n):
                    if qt != 0 and p + 2 < kp_n:
                        pts[p + 2] = make_pair(qt, p + 2, ptp)
                    pt2 = pts.pop(p)
                    for cj in range(4):
                        nc.tensor.matmul(
                            oT_ps[:, cj, :],
                            lhsT=vN[:, 2 * p:2 * p + 2,
                                    cj * 128:(cj + 1) * 128],
                            rhs=pt2[:],
                            perf_mode=DR,
                            start=(p == 0), stop=(p == kp_n - 1))
                    nc.tensor.matmul(
                        oT_ps[:, 4, :],
                        lhsT=ones8,
                        rhs=pt2[:],
                        perf_mode=DR,
                        start=(p == 0), stop=(p == kp_n - 1))
                srow, oT8 = epilogue_a(qt, oT_ps)
                pending = (qt, xr_t, srow, oT8)
            epilogue_b(*pending)

    nc.compile()
    return nc


def prep_host_inputs(x, ln_g, ln_b, Wq, bq, Wk, bk, Wv, bv, Wp, bp,
                     n_tok=N_TOK, nq=NQ, ncores=NCORES, nbatch=B):
    """Fold LN affine + linear biases on the host; build per-core maps."""
    f32 = np.float32
    x = np.asarray(x, f32)
    g = np.asarray(ln_g, f32)
    b = np.asarray(ln_b, f32)
    Wq = np.asarray(Wq, f32); Wk = np.asarray(Wk, f32)
    Wv = np.asarray(Wv, f32); Wp = np.asarray(Wp, f32)
    bq = np.asarray(bq, f32); bk = np.asarray(bk, f32)
    bv = np.asarray(bv, f32); bp = np.asarray(bp, f32)

    wq_e = g[:, None] * Wq
    bq_e = b @ Wq + bq
    wk_e = g[:, None] * Wk
    bk_e = b @ Wk + bk
    wv_e = g[:, None] * Wv
    bv_e = b @ Wv + bv
    resid_const = bv_e @ Wp + bp    # [C]

    ci = C // 128
    bq_pp = np.ascontiguousarray(bq_e.reshape(ci, 128).T).astype(f32)
    bk_pp = np.ascontiguousarray(bk_e.reshape(ci, 128).T).astype(f32)
    ident = np.eye(128, dtype=ml_dtypes.bfloat16)
    onesv = np.ones((128, 2, 128), dtype=NPF8)

    shared = dict(
        wq=wq_e.astype(NPF8), wk=wk_e.astype(NPF8),
        wv=wv_e.astype(NPF8), wp=(Wp * WPSCALE).astype(NPF8),
        bq=bq_pp, bk=bk_pp, ident=ident, ones=onesv,
    )

    xf = x.reshape(-1, C)  # flattened tokens, nbatch * n_tok rows
    halves = ncores // nbatch
    in_maps = []
    for core in range(ncores):
        bidx, half = divmod(core, halves)
        xb = xf[bidx * n_tok:(bidx + 1) * n_tok]
        if half:
            xp = np.ascontiguousarray(
                np.concatenate([xb[half * nq:], xb[:half * nq]], axis=0))
        else:
            xp = xb
        xr = (xp[:nq] + resid_const).astype(f32)
        m = dict(shared)
        m["x"] = np.ascontiguousarray(xp)
        m["xr"] = np.ascontiguousarray(xr)
        in_maps.append(m)
    return in_maps


_PROG = None


def _get_prog():
    global _PROG
    if _PROG is None:
        _PROG = build_program()
    return _PROG


def kernel(x, ln_g, ln_b, Wq, bq, Wk, bk, Wv, bv, Wp, bp, _trace=False,
           _tmpdir=None):
    global LAST_EXEC_NS, LAST_RESULT
    nc = _get_prog()
    in_maps = prep_host_inputs(x, ln_g, ln_b, Wq, bq, Wk, bk, Wv, bv, Wp, bp)
    res = run_bass_kernel_spmd(nc, in_maps, list(range(NCORES)), trace=_trace,
                               tmpdir=_tmpdir)
    LAST_EXEC_NS = res.exec_time_ns
    LAST_RESULT = res
    y = np.empty((B, N_TOK, C), np.float32)
    halves = NCORES // B
    for core in range(NCORES):
        bidx, half = divmod(core, halves)
        y[bidx, half * NQ:(half + 1) * NQ] = res.results[core]["y"]
    return y.reshape(B, Hh, Ww, C)


# revision 18
# speedup vs baseline: 1.0151x; 1.0084x over previous
"""Trainium2 Bass kernel for nn_AttentionBlock (B=4, H=W=64, C=512).

Strategy (8 cores, no collectives):
  - 2 cores per batch image; each core handles 2048 of the 4096 queries.
  - Key/token order is permuted per core so that each core's OWN query rows
    are tokens 0..2047 of its private x copy (softmax is invariant to key
    permutation as long as K and V use the same order).
  - All GEMMs run in fp8e4 with MatmulPerfMode.DoubleRow (2x bf16 rate):
    lhsT/rhs carry [128, 2, *] channel- or key-chunk pairs so each matmul
    contracts 256 elements. (Dual-fp8 LDWEIGHTS requires a 128-column
    stationary, hence the full-width ones matrix for the sums plane.)
  - Per core: LayerNorm (bn_stats, batched 2 tiles per Ln/Exp), transpose hn
    to channel-major hfT (bf16 PE transpose, fp8 cast on the PSUM->SBUF
    copy), Q^T/K^T (channel-major) + V (token-major) projections in fp8:
        S^T[k,q] = K^T.T @ Q^T     (PSUM fp32)
        P^T = exp(S^T/sqrt(C)-2.9) (ACT, scale+shift folded into the table)
        O^T[c,q] += V.T-pair @ P^T (PSUM planes 0-3, no output transpose)
        sums[q]  += ones.T @ P^T   (PSUM plane 4)
        y = (O^T fp8) proj via 4*Wp back to [q,c]; y *= 1/(4*sums);
        out = y + x + const-biases
  - The scores+exp for the first query tile are computed DURING stage A/B
    (kT/vN chunks become valid in token order), overlapping the copy-bound
    projection phase with ACT exp work; their P^T tiles are cached in SBUF.
  - Epilogue split: sums-row + O^T fp8 casts (ACT) right after the last PV;
    transposes/recip/y-proj deferred past the next tile's first score pairs
    so the PE queue never drains.
  - LN gamma/beta are folded into the QKV weights/biases on the host;
    bv/bp biases are folded into the residual input xr on the host; the
    softmax 1/sqrt(C) scale is applied by the ACT exp instruction.
"""

import os
import sys

import numpy as np
import ml_dtypes

try:
    import concourse.bass as bass
except ImportError:  # pragma: no cover - fresh-dir fallback
    for _p in ("/opt/trn_rl_repo", "/root/.axon_site/_ro/trn_rl_repo"):
        if os.path.isdir(_p) and _p not in sys.path:
            sys.path.insert(0, _p)
    import concourse.bass as bass

import concourse.bacc as bacc
import concourse.tile as tile
from concourse import mybir
from concourse.bass_utils import run_bass_kernel_spmd

F32 = mybir.dt.float32
BF16 = mybir.dt.bfloat16
F8 = mybir.dt.float8e4
AF = mybir.ActivationFunctionType
ALU = mybir.AluOpType
DR = mybir.MatmulPerfMode.DoubleRow
NPF8 = ml_dtypes.float8_e4m3

B, Hh, Ww, C = 4, 64, 64, 512
N_TOK = Hh * Ww          # 4096 tokens per image
NCORES = 8
NQ = N_TOK * B // NCORES  # 2048 queries per core
LN_EPS = 1e-3
CI = C // 128             # 4 channel chunks
SSCALE = 1.0 / float(np.sqrt(np.float32(C)))  # softmax scale, applied in exp
# exp(S*scale + ESHIFT): keeps P <= ~30 and O^T <= ~150 (fp8e4 max 240),
# so the O^T PSUM->SBUF copy is a pure cast. The extra ln(4) is undone by
# 4*Wp and the 4x srow scale (normalization is scale-invariant).
ESHIFT = -(1.5 + float(np.log(4.0)))
WPSCALE = 4.0

LAST_EXEC_NS = None
LAST_RESULT = None


def build_program(n_tok=N_TOK, nq=NQ):
    """Build the per-core Bass program (identical across cores)."""
    assert n_tok % 1024 == 0 and nq % 512 == 0
    nt_tiles = n_tok // 512   # n-tiles for K/V over all tokens
    qt_tiles = nq // 512      # q-tiles for this core's queries
    kc_n = n_tok // 128       # key chunks
    kp_n = kc_n // 2          # key chunk pairs

    nc = bacc.Bacc()
    if os.environ.get("BASS_CACHE_BUST"):
        nc.dram_tensor(f"cachebust_{os.environ['BASS_CACHE_BUST']}", [1, 1], F32)
    x_d = nc.dram_tensor("x", [n_tok, C], F32, kind="ExternalInput")
    xr_d = nc.dram_tensor("xr", [nq, C], F32, kind="ExternalInput")
    wq_d = nc.dram_tensor("wq", [C, C], F8, kind="ExternalInput")
    wk_d = nc.dram_tensor("wk", [C, C], F8, kind="ExternalInput")
    wvp_d = nc.dram_tensor("wvp", [C, C], F8, kind="ExternalInput")
    bq_d = nc.dram_tensor("bq", [128, CI], F32, kind="ExternalInput")
    bk_d = nc.dram_tensor("bk", [128, CI], F32, kind="ExternalInput")
    id_d = nc.dram_tensor("ident", [128, 128], BF16, kind="ExternalInput")
    on_d = nc.dram_tensor("ones", [128, 2, 128], F8, kind="ExternalInput")
    y_d = nc.dram_tensor("y", [nq, C], F32, kind="ExternalOutput")

    # token index mapping: tok = tile*512 + k*128 + p  (p = partition)
    x_re = x_d[:].rearrange("(t k p) c -> p t k c", p=128, k=4)
    xr_re = xr_d[:].rearrange("(t k p) c -> p t k c", p=128, k=4)
    y_re = y_d[:].rearrange("(t k p) c -> p t k c", p=128, k=4)

    from contextlib import ExitStack

    with ExitStack() as ctx:
        tc = ctx.enter_context(tile.TileContext(nc))
        consts = ctx.enter_context(tc.tile_pool(name="consts", bufs=1))
        big = ctx.enter_context(tc.tile_pool(name="big", bufs=1))
        work = ctx.enter_context(tc.tile_pool(name="work", bufs=3))
        stat = ctx.enter_context(tc.tile_pool(name="stat", bufs=4))
        ptp = ctx.enter_context(tc.tile_pool(name="ptp", bufs=4))
        ptc = ctx.enter_context(tc.tile_pool(name="ptc", bufs=kp_n))
        epi = ctx.enter_context(tc.tile_pool(name="epi", bufs=3))
        psS = ctx.enter_context(tc.tile_pool(name="psS", bufs=3, space="PSUM"))

        # ---- first x tile + transpose identity first: they gate the
        # ---- pipeline head; weights are only needed a few µs later.
        x_t0 = work.tile([128, 4, C], F32, tag="x", bufs=6)
        for k in range(4):
            nc.sync.dma_start(out=x_t0[:, k, :], in_=x_re[:, 0, k, :])
        ident = consts.tile([128, 128], BF16)
        nc.sync.dma_start(out=ident, in_=id_d[:])

        # ---- constants ----
        wq_sb = consts.tile([128, CI, C], F8)
        nc.sync.dma_start(out=wq_sb, in_=wq_d[:].rearrange("(ci p) co -> p ci co", p=128))
        wk_sb = consts.tile([128, CI, C], F8)
        nc.sync.dma_start(out=wk_sb, in_=wk_d[:].rearrange("(ci p) co -> p ci co", p=128))
        wvp_sb = consts.tile([128, CI, C], F8)
        nc.sync.dma_start(out=wvp_sb, in_=wvp_d[:].rearrange("(ci p) co -> p ci co", p=128))
        bq_sb = consts.tile([128, CI], F32)
        nc.sync.dma_start(out=bq_sb, in_=bq_d[:])
        bk_sb = consts.tile([128, CI], F32)
        nc.sync.dma_start(out=bk_sb, in_=bk_d[:])
        ones8 = consts.tile([128, 2, 128], F8)
        nc.sync.dma_start(out=ones8, in_=on_d[:])
        eps_sb = consts.tile([128, 1], F32)
        nc.vector.memset(eps_sb, LN_EPS)
        shf_sb = consts.tile([128, 1], F32)
        nc.vector.memset(shf_sb, ESHIFT)

        # ---- persistent activations (all fp8, channel pairs sliceable) ----
        hfT = big.tile([128, CI, n_tok], F8)     # normalized x, channel-major
        kT = big.tile([128, CI, n_tok], F8)      # K^T, channel-major
        hN = big.tile([128, kc_n, C], F8)        # hn, token-major chunks
        qT = big.tile([128, CI, nq], F8)         # Q^T, channel-major

        # scores + exp for one key chunk; pipelined ahead of PV use
        def st_exp(qt, kc, pt2, plane):
            s_ps = psS.tile([128, 512], F32, tag="st",
                            name=f"s_ps_{qt}_{kc}")
            for ip in range(CI // 2):
                nc.tensor.matmul(
                    s_ps,
                    lhsT=kT[:, 2 * ip:2 * ip + 2,
                            kc * 128:(kc + 1) * 128],
                    rhs=qT[:, 2 * ip:2 * ip + 2,
                           qt * 512:(qt + 1) * 512],
                    perf_mode=DR,
                    start=(ip == 0), stop=(ip == CI // 2 - 1))
            nc.scalar.activation(out=pt2[:, plane, :], in_=s_ps,
                                 func=AF.Exp, scale=SSCALE,
                                 bias=shf_sb)

        def make_pair(qt, p, pool):
            pt2 = pool.tile([128, 2, 512], F8, tag="pt",
                            name=f"pt_{qt}_{p}")
            st_exp(qt, 2 * p, pt2, 0)
            st_exp(qt, 2 * p + 1, pt2, 1)
            return pt2

        # ========= Stage A+B: LN, transpose, projections; the scores+exp
        # ========= for query tile 0 are interleaved as kT chunks land.
        pt0_cache = []
        xtiles = {0: x_t0}

        def fetch_x(t):
            if t not in xtiles and t < nt_tiles:
                xt = work.tile([128, 4, C], F32, tag="x", bufs=6,
                               name=f"x_{t}")
                nc.sync.dma_start(out=xt, in_=x_re[:, t, :, :])
                xtiles[t] = xt

        fetch_x(1)

        def emit_stats(tp):
            """bn_stats + DVE fast-rsqrt for tile pair tp -> (rstd8, b8)."""
            xts = [xtiles[2 * tp], xtiles[2 * tp + 1]]
            mv8 = stat.tile([128, 8, 2], F32, tag="mv", name=f"mv_{tp}")
            for ti in range(2):
                for k in range(4):
                    stats = stat.tile([128, 6], F32, tag="bnst")
                    nc.vector.bn_stats(out=stats, in_=xts[ti][:, k, :])
                    nc.vector.bn_aggr(out=mv8[:, 4 * ti + k, :], in_=stats)
            # rstd = rsqrt(var+eps) entirely on DVE (bitcast seed + two
            # Newton steps): ACT Ln/Sqrt would force a 1.3us activation-
            # table swap away from the exp set.
            I32 = mybir.dt.int32
            veps = stat.tile([128, 8], F32, tag="veps")
            nc.vector.tensor_scalar_add(out=veps, in0=mv8[:, :, 1],
                                        scalar1=LN_EPS)
            yb = stat.tile([128, 8], I32, tag="yb")
            nc.vector.tensor_scalar(out=yb,
                                    in0=veps[:].bitcast(I32),
                                    scalar1=1, scalar2=None,
                                    op0=ALU.logical_shift_right)
            y0b = stat.tile([128, 8], I32, tag="y0b")
            nc.vector.tensor_scalar(out=y0b, in0=yb,
                                    scalar1=0x5f3759df, scalar2=-1,
                                    op0=ALU.subtract, op1=ALU.mult)
            t1 = stat.tile([128, 8], F32, tag="nt1")
            nc.vector.tensor_tensor(out=t1, in0=y0b[:].bitcast(F32),
                                    in1=y0b[:].bitcast(F32), op=ALU.mult)
            t2 = stat.tile([128, 8], F32, tag="nt2")
            nc.vector.tensor_tensor(out=t2, in0=t1, in1=veps, op=ALU.mult)
            t3 = stat.tile([128, 8], F32, tag="nt3")
            nc.vector.tensor_scalar(out=t3, in0=t2, scalar1=-0.5,
                                    scalar2=1.5, op0=ALU.mult, op1=ALU.add)
            y1 = stat.tile([128, 8], F32, tag="y1")
            nc.vector.tensor_tensor(out=y1, in0=y0b[:].bitcast(F32),
                                    in1=t3, op=ALU.mult)
            u1 = stat.tile([128, 8], F32, tag="nu1")
            nc.vector.tensor_tensor(out=u1, in0=y1, in1=y1, op=ALU.mult)
            u2 = stat.tile([128, 8], F32, tag="nu2")
            nc.vector.tensor_tensor(out=u2, in0=u1, in1=veps, op=ALU.mult)
            u3 = stat.tile([128, 8], F32, tag="nu3")
            nc.vector.tensor_scalar(out=u3, in0=u2, scalar1=-0.5,
                                    scalar2=1.5, op0=ALU.mult, op1=ALU.add)
            rstd8 = stat.tile([128, 8], F32, tag="rstd", name=f"rstd_{tp}")
            nc.vector.tensor_tensor(out=rstd8, in0=y1, in1=u3, op=ALU.mult)
            # hn is applied on ACT as Identity(x*rstd + (-mu*rstd))
            mr = stat.tile([128, 8], F32, tag="mr")
            nc.vector.tensor_tensor(out=mr, in0=mv8[:, :, 0], in1=rstd8,
                                    op=ALU.mult)
            b8 = stat.tile([128, 8], F32, tag="b8", name=f"b8_{tp}")
            nc.vector.tensor_scalar_mul(out=b8, in0=mr, scalar1=-1.0)
            return rstd8, b8

        ln_aff = {}
        with tc.tile_pool(name="psAB", bufs=4, space="PSUM") as psAB:
            for tp in range(nt_tiles // 2):
                # prefetch the NEXT pair's x tiles so the stats chain for
                # tile pair tp+1 never stalls the PE at the boundary
                fetch_x(2 * tp + 2)
                fetch_x(2 * tp + 3)
                if tp == 0:
                    ln_aff[0] = emit_stats(0)
                rstd8, b8 = ln_aff.pop(tp)
                for ti in range(2):
                    t = 2 * tp + ti
                    x_t = xtiles[t]
                    # emit next pair's stats midway: its x DMAs have landed
                    # and the DVE reaches it before the boundary
                    if ti == 1 and tp + 1 < nt_tiles // 2:
                        ln_aff[tp + 1] = emit_stats(tp + 1)
                    for k in range(4):
                        chunk = t * 4 + k
                        idx = 4 * ti + k
                        hn = work.tile([128, C], BF16, tag="hn", bufs=4)
                        nc.scalar.activation(out=hn, in_=x_t[:, k, :],
                                             func=AF.Identity,
                                             scale=rstd8[:, idx:idx + 1],
                                             bias=b8[:, idx:idx + 1])
                        tr_ps = psAB.tile([128, CI, 128], BF16, tag="ps")
                        for j in range(CI):
                            nc.tensor.transpose(tr_ps[:, j, :],
                                                hn[:, j * 128:(j + 1) * 128],
                                                ident)
                        # fp8 cast on the PSUM->SBUF copy
                        nc.vector.tensor_copy(
                            out=hfT[:, :, chunk * 128:(chunk + 1) * 128],
                            in_=tr_ps)
                        # token-major fp8 copy of hn: V/out projections are
                        # fused into Wv@Wp on the host, so P@hn is
                        # accumulated directly (no V projection on device)
                        if k % 2 == 0:
                            nc.scalar.copy(out=hN[:, chunk, :], in_=hn)
                        else:
                            nc.vector.tensor_copy(out=hN[:, chunk, :],
                                                  in_=hn)

                    # K^T columns for this tile
                    for j in range(CI):
                        k_ps = psAB.tile([128, 512], F32, tag="ps")
                        for ip in range(CI // 2):
                            nc.tensor.matmul(
                                k_ps,
                                lhsT=wk_sb[:, 2 * ip:2 * ip + 2,
                                           j * 128:(j + 1) * 128],
                                rhs=hfT[:, 2 * ip:2 * ip + 2,
                                        t * 512:(t + 1) * 512],
                                perf_mode=DR,
                                start=(ip == 0), stop=(ip == CI // 2 - 1))
                        if j % 2 == 0:
                            nc.scalar.activation(
                                out=kT[:, j, t * 512:(t + 1) * 512],
                                in_=k_ps, func=AF.Identity,
                                bias=bk_sb[:, j:j + 1])
                        else:
                            nc.vector.tensor_scalar_add(
                                out=kT[:, j, t * 512:(t + 1) * 512],
                                in0=k_ps, scalar1=bk_sb[:, j:j + 1])

                    # Q^T columns (only for this core's query range)
                    if t < qt_tiles:
                        for j in range(CI):
                            q_ps = psAB.tile([128, 512], F32, tag="ps")
                            for ip in range(CI // 2):
                                nc.tensor.matmul(
                                    q_ps,
                                    lhsT=wq_sb[:, 2 * ip:2 * ip + 2,
                                               j * 128:(j + 1) * 128],
                                    rhs=hfT[:, 2 * ip:2 * ip + 2,
                                            t * 512:(t + 1) * 512],
                                    perf_mode=DR,
                                    start=(ip == 0), stop=(ip == CI // 2 - 1))
                            if j % 2 == 0:
                                nc.scalar.activation(
                                    out=qT[:, j, t * 512:(t + 1) * 512],
                                    in_=q_ps, func=AF.Identity,
                                    bias=bq_sb[:, j:j + 1])
                            else:
                                nc.vector.tensor_scalar_add(
                                    out=qT[:, j, t * 512:(t + 1) * 512],
                                    in0=q_ps, scalar1=bq_sb[:, j:j + 1])

                # prebuild query-tile-0 score pairs for this tp's kc range
                for p in range(4 * tp, 4 * tp + 4):
                    pt0_cache.append(make_pair(0, p, ptc))

        # ================= Stage C: attention ============================
        with tc.tile_pool(name="psO", bufs=1, space="PSUM") as psO:

            # part A: drain O^T/sums out of PSUM (ACT); emitted right after
            # the last PV so the next tile's oT_ps reallocation is safe.
            def epilogue_a(qt, oT_ps):
                srow = epi.tile([1, 512], BF16, tag="srow",
                                name=f"srow_{qt}")
                # 4x so recip = 1/(4*sums) matches y_ps = o8 @ (4*Wp)
                nc.scalar.mul(srow, oT_ps[0:1, 4, :], WPSCALE)
                oT8 = epi.tile([128, 4, C], F8, tag="ot8", name=f"ot8_{qt}")
                for cj in range(4):  # pure casts: one per PSUM bank
                    nc.scalar.copy(out=oT8[:, cj, :], in_=oT_ps[:, cj, :])
                return srow, oT8

            # part B: transposes/recip/y-projection; deferred past the next
            # tile's first score pairs so PE always has queued work.
            def epilogue_b(qt, xr_t, srow, oT8):
                # bf16 PSUM writes need 4-byte alignment -> stride-2 columns
                st4 = psS.tile([128, 4, 2], BF16, tag="st",
                               name=f"st4_{qt}")
                for i in range(4):
                    nc.tensor.transpose(st4[:, i, 0:1],
                                        srow[0:1, i * 128:(i + 1) * 128],
                                        ident[0:1, 0:1])
                recip = stat.tile([128, 4], F32, tag="recip",
                                  name=f"recip_{qt}")
                nc.vector.reciprocal(out=recip, in_=st4[:, :, 0])
                for qc in range(4):
                    y_ps = psS.tile([128, C], F32, tag="st",
                                    name=f"y_ps_{qt}_{qc}")
                    for ip in range(CI // 2):
                        nc.tensor.matmul(
                            y_ps,
                            lhsT=oT8[:, 2 * ip:2 * ip + 2,
                                     qc * 128:(qc + 1) * 128],
                            rhs=wvp_sb[:, 2 * ip:2 * ip + 2, :],
                            perf_mode=DR,
                            start=(ip == 0), stop=(ip == CI // 2 - 1))
                    y_sb = epi.tile([128, C], F32, tag="ysb")
                    nc.vector.scalar_tensor_tensor(
                        out=y_sb, in0=y_ps, scalar=recip[:, qc:qc + 1],
                        in1=xr_t[:, qc, :], op0=ALU.mult, op1=ALU.add)
                    nc.sync.dma_start(out=y_re[:, qt, qc, :], in_=y_sb)

            pending = None
            for qt in range(qt_tiles):
                xr_t = work.tile([128, 4, C], F32, tag="xr")
                nc.sync.dma_start(out=xr_t, in_=xr_re[:, qt, :, :])
                if qt == 0:
                    pts = dict(enumerate(pt0_cache))
                else:
                    pts = {0: make_pair(qt, 0, ptp),
                           1: make_pair(qt, 1, ptp)}
                if pending is not None:
                    epilogue_b(*pending)
                    pending = None
                # planes 0-3: O^T[cj]; plane 4: per-query sums (replicated)
                oT_ps = psO.tile([128, 5, C], F32, tag="o", name=f"o_{qt}")
                for p in range(kp_n):
                    if qt != 0 and p + 2 < kp_n:
                        pts[p + 2] = make_pair(qt, p + 2, ptp)
                    pt2 = pts.pop(p)
                    for cj in range(4):
                        nc.tensor.matmul(
                            oT_ps[:, cj, :],
                            lhsT=hN[:, 2 * p:2 * p + 2,
                                    cj * 128:(cj + 1) * 128],
                            rhs=pt2[:],
                            perf_mode=DR,
                            start=(p == 0), stop=(p == kp_n - 1))
                    nc.tensor.matmul(
                        oT_ps[:, 4, :],
                        lhsT=ones8,
                        rhs=pt2[:],
                        perf_mode=DR,
                        start=(p == 0), stop=(p == kp_n - 1))
                srow, oT8 = epilogue_a(qt, oT_ps)
                pending = (qt, xr_t, srow, oT8)
            epilogue_b(*pending)

    nc.compile()
    return nc


def prep_host_inputs(x, ln_g, ln_b, Wq, bq, Wk, bk, Wv, bv, Wp, bp,
                     n_tok=N_TOK, nq=NQ, ncores=NCORES, nbatch=B):
    """Fold LN affine + linear biases on the host; build per-core maps."""
    f32 = np.float32
    x = np.asarray(x, f32)
    g = np.asarray(ln_g, f32)
    b = np.asarray(ln_b, f32)
    Wq = np.asarray(Wq, f32); Wk = np.asarray(Wk, f32)
    Wv = np.asarray(Wv, f32); Wp = np.asarray(Wp, f32)
    bq = np.asarray(bq, f32); bk = np.asarray(bk, f32)
    bv = np.asarray(bv, f32); bp = np.asarray(bp, f32)

    wq_e = g[:, None] * Wq
    bq_e = b @ Wq + bq
    wk_e = g[:, None] * Wk
    bk_e = b @ Wk + bk
    wv_e = g[:, None] * Wv
    bv_e = b @ Wv + bv
    resid_const = bv_e @ Wp + bp    # [C]

    ci = C // 128
    bq_pp = np.ascontiguousarray(bq_e.reshape(ci, 128).T).astype(f32)
    bk_pp = np.ascontiguousarray(bk_e.reshape(ci, 128).T).astype(f32)
    ident = np.eye(128, dtype=ml_dtypes.bfloat16)
    onesv = np.ones((128, 2, 128), dtype=NPF8)

    wvp = wv_e @ Wp
    shared = dict(
        wq=wq_e.astype(NPF8), wk=wk_e.astype(NPF8),
        wvp=(wvp * WPSCALE).astype(NPF8),
        bq=bq_pp, bk=bk_pp, ident=ident, ones=onesv,
    )

    xf = x.reshape(-1, C)  # flattened tokens, nbatch * n_tok rows
    halves = ncores // nbatch
    in_maps = []
    for core in range(ncores):
        bidx, half = divmod(core, halves)
        xb = xf[bidx * n_tok:(bidx + 1) * n_tok]
        if half:
            xp = np.ascontiguousarray(
                np.concatenate([xb[half * nq:], xb[:half * nq]], axis=0))
        else:
            xp = xb
        xr = (xp[:nq] + resid_const).astype(f32)
        m = dict(shared)
        m["x"] = np.ascontiguousarray(xp)
        m["xr"] = np.ascontiguousarray(xr)
        in_maps.append(m)
    return in_maps


_PROG = None


def _get_prog():
    global _PROG
    if _PROG is None:
        _PROG = build_program()
    return _PROG


def kernel(x, ln_g, ln_b, Wq, bq, Wk, bk, Wv, bv, Wp, bp, _trace=False,
           _tmpdir=None):
    global LAST_EXEC_NS, LAST_RESULT
    nc = _get_prog()
    in_maps = prep_host_inputs(x, ln_g, ln_b, Wq, bq, Wk, bk, Wv, bv, Wp, bp)
    res = run_bass_kernel_spmd(nc, in_maps, list(range(NCORES)), trace=_trace,
                               tmpdir=_tmpdir)
    LAST_EXEC_NS = res.exec_time_ns
    LAST_RESULT = res
    y = np.empty((B, N_TOK, C), np.float32)
    halves = NCORES // B
    for core in range(NCORES):
        bidx, half = divmod(core, halves)
        y[bidx, half * NQ:(half + 1) * NQ] = res.results[core]["y"]
    return y.reshape(B, Hh, Ww, C)
